# revision 6
# baseline (speedup 1.0000x reference)
"""Trainium2 Bass kernel for nn_Cluster_62294205661436 (GNN message passing
cluster module). Data-parallel over graphs: 64 disjoint graphs of 64 nodes,
8 graphs per NeuronCore. All dense math (adjacency build via one-hot matmuls,
k-hop propagation, 4 attention MLPs, edge softmax, S^T A S) runs on device in
f32; the host only shards inputs, computes the per-graph top-k permutation
from device logits, and assembles the block-diagonal outputs.
"""
import os
import numpy as np

import concourse.bass as bass
import concourse.bacc as bacc
import concourse.mybir as mybir
import concourse.tile as tile
from concourse.bass_utils import run_bass_kernel_spmd

F32 = mybir.dt.float32
AF = mybir.ActivationFunctionType
OP = mybir.AluOpType
AX = mybir.AxisListType

# problem constants (hardcoded per contest rules)
B = 64          # graphs total
N_PER = 64      # nodes per graph
H = 128         # feature dim
EPG = 2048      # edges per graph
NCORES = 8
G = 8           # graphs per core
NODES = G * N_PER       # 512 nodes per core
NCHUNK = EPG // 128     # 16 edge chunks of 128 per graph
KK = 52         # ceil(0.8 * 64)
ALPHA = 0.01    # leaky relu slope

_NC_CACHE = {}


def _attention(nc, sb, ps, consts, kv_fm, q_fm, W, a, f_out, l_out):
    """Emit one attention head: logits + per-graph softmax.

    kv_fm/q_fm: [128, 512] feature-major SBUF tiles.
    Writes f_out [1,512] (softmax weights) and l_out [1,512] (pre-softmax
    logits, after final Lrelu).
    """
    Wk, W1, W2, W3 = W
    # a = kv @ Wk  (feature-major: out = Wk^T @ kv_fm)
    a_ps = ps.tile([128, 512], F32, tag="mlp")
    nc.tensor.matmul(a_ps[:], Wk[:, a * 128:(a + 1) * 128], kv_fm[:],
                     start=True, stop=True)
    a_sb = sb.tile([128, 512], F32, tag=f"att_a")
    nc.scalar.copy(a_sb[:], a_ps[:])
    # h blocks: [a, q, a-q, a*q]
    d_sb = sb.tile([128, 512], F32, tag="att_d")
    m_sb = sb.tile([128, 512], F32, tag="att_m")
    nc.vector.tensor_tensor(d_sb[:], a_sb[:], q_fm[:], OP.subtract)
    nc.vector.tensor_tensor(m_sb[:], a_sb[:], q_fm[:], OP.mult)
    hblk = [a_sb, q_fm, d_sb, m_sb]
    # layer 1: 512 -> 256 (two 128-col chunks)
    h1 = []
    for mc in range(2):
        h1_ps = ps.tile([128, 512], F32, tag="mlp")
        for kc in range(4):
            col = ((a * 4 + kc) * 2 + mc) * 128
            nc.tensor.matmul(h1_ps[:], W1[:, col:col + 128], hblk[kc][:],
                             start=(kc == 0), stop=(kc == 3))
        h1_sb = sb.tile([128, 512], F32, tag=f"att_h1_{mc}")
        _lrelu(nc, sb, h1_sb, h1_ps, f"h1_{mc}")
        h1.append(h1_sb)
    # layer 2: 256 -> 128
    h2_ps = ps.tile([128, 512], F32, tag="mlp")
    for kc in range(2):
        col = (a * 2 + kc) * 128
        nc.tensor.matmul(h2_ps[:], W2[:, col:col + 128], h1[kc][:],
                         start=(kc == 0), stop=(kc == 1))
    h2_sb = sb.tile([128, 512], F32, tag="att_h2")
    _lrelu(nc, sb, h2_sb, h2_ps, "h2")
    # layer 3: 128 -> 1
    l_ps = ps.tile([1, 512], F32, tag="mlp")
    nc.tensor.matmul(l_ps[:], W3[:, a:a + 1], h2_sb[:], start=True, stop=True)
    _lrelu(nc, sb, l_out, l_ps, "lg")
    # per-graph softmax over 64-node segments (with max subtraction)
    l3 = l_out[:].rearrange("p (g n) -> p g n", n=N_PER)
    m8 = sb.tile([1, G], F32, tag="att_m8")
    nc.vector.tensor_reduce(m8[:], l3, AX.X, OP.max)
    lm = sb.tile([1, 512], F32, tag="att_lm")
    nc.vector.tensor_tensor(
        lm[:].rearrange("p (g n) -> p g n", n=N_PER), l3,
        m8[:, :, None].broadcast_to([1, G, N_PER]), OP.subtract)
    e_sb = sb.tile([1, 512], F32, tag="att_e")
    nc.scalar.activation(e_sb[:], lm[:], AF.Exp)
    d8 = sb.tile([1, G], F32, tag="att_d8")
    nc.vector.tensor_reduce(d8[:], e_sb[:].rearrange("p (g n) -> p g n", n=N_PER),
                            AX.X, OP.add)
    r8 = sb.tile([1, G], F32, tag="att_r8")
    _recip(nc, sb, r8, d8, "att_r8")
    nc.vector.tensor_tensor(
        f_out[:].rearrange("p (g n) -> p g n", n=N_PER),
        e_sb[:].rearrange("p (g n) -> p g n", n=N_PER),
        r8[:, :, None].broadcast_to([1, G, N_PER]), OP.mult)




def _lrelu(nc, sb, out, in_, tag):
    """Exact leaky relu: out = max(in, 0.01*in) (== jax.nn.leaky_relu)."""
    t = sb.tile(list(out.shape), F32, tag=tag + "_lr", name="lrt")
    nc.vector.tensor_scalar(t[:], in_[:], ALPHA, None, OP.mult)
    nc.vector.tensor_tensor(out[:], in_[:], t[:], OP.max)


def _recip(nc, sb, out, d, tag):
    """out = 1/d with one Newton step on top of the DVE reciprocal."""
    shape = list(out.shape)
    r0 = sb.tile(shape, F32, tag=tag + "_r0", name="r0t")
    nc.vector.reciprocal(r0[:], d[:])
    t = sb.tile(shape, F32, tag=tag + "_t", name="tt")
    nc.vector.tensor_tensor(t[:], d[:], r0[:], OP.mult)
    # t2 = 2 - t  (chain: *-1, +2)
    nc.vector.tensor_scalar(t[:], t[:], -1.0, 2.0, OP.mult, OP.add)
    nc.vector.tensor_tensor(out[:], r0[:], t[:], OP.mult)


def _rsqrt(nc, sb, out, d, tag):
    """out = 1/sqrt(d) via ACT sqrt + DVE recip + one rsqrt Newton step."""
    shape = list(out.shape)
    sq = sb.tile(shape, F32, tag=tag + "_sq", name="sqt")
    nc.scalar.activation(sq[:], d[:], AF.Sqrt)
    y0 = sb.tile(shape, F32, tag=tag + "_y0", name="y0t")
    nc.vector.reciprocal(y0[:], sq[:])
    t = sb.tile(shape, F32, tag=tag + "_t", name="tt2")
    nc.vector.tensor_tensor(t[:], d[:], y0[:], OP.mult)
    nc.vector.tensor_tensor(t[:], t[:], y0[:], OP.mult)
    # t = 1.5 - 0.5*t
    nc.vector.tensor_scalar(t[:], t[:], -0.5, 1.5, OP.mult, OP.add)
    nc.vector.tensor_tensor(out[:], y0[:], t[:], OP.mult)


def _build_nc(debug=False):
    nc = bacc.Bacc("TRN2", target_bir_lowering=False, debug=False)

    # ------------- I/O -------------
    inp = {}
    for name, shape in [
        ("x_nm", [N_PER, G * H]),      # node-major, graph g at cols g*128..
        ("x_fm", [H, NODES]),          # feature-major
        ("qt_fm", [H, NODES]),
        ("srcL", [128, G * NCHUNK]),   # local src idx, col = g*16+k
        ("dstL", [128, G * NCHUNK]),
        ("wE", [128, G * NCHUNK]),
        ("iota8", [128, 8 * N_PER]),   # rows = tile(arange(64), 8)
        ("i64x2", [N_PER, 128]),       # [I64 | I64]
        ("idn", [128, 128]),           # I128
        ("ones1", [1, 128]),
        ("ones64", [N_PER, 1]),
        ("Wk_all", [128, 4 * 128]),
        ("W1_all", [128, 4 * 4 * 2 * 128]),
        ("W2_all", [128, 4 * 2 * 128]),
        ("W3_all", [128, 4]),
        ("lin_all", [128, 2 * 128]),
    ]:
        inp[name] = nc.dram_tensor(name, shape, F32, kind="ExternalInput")

    out = {}
    for name, shape in [
        ("xs_out", [N_PER, G * H]),
        ("at2_out", [G, N_PER, N_PER]),
        ("l1_out", [1, NODES]),
        ("l2_out", [1, NODES]),
        ("score_out", [1, NODES]),
    ]:
        out[name] = nc.dram_tensor(name, shape, F32, kind="ExternalOutput")
    if debug:
        for name, shape in [
            ("dbg_cw", [N_PER, G * 128]),     # [Cnt|Wsum] per graph
            ("dbg_a", [N_PER, G * N_PER]),    # A per graph
            ("dbg_s", [N_PER, G * N_PER]),    # S per graph
            ("dbg_xq", [H, NODES]),
            ("dbg_xc", [H, NODES]),
            ("dbg_f1", [1, NODES]),
            ("dbg_f2", [1, NODES]),
            ("dbg_dinv", [1, NODES]),
        ]:
            out[name] = nc.dram_tensor(name, shape, F32, kind="ExternalOutput")

    with tile.TileContext(nc) as tc:
        with (
            tc.tile_pool(name="const", bufs=1) as cp,
            tc.tile_pool(name="sb", bufs=2) as sb,
            tc.tile_pool(name="pers", bufs=1) as pp,
            tc.tile_pool(name="ps", bufs=2, space="PSUM") as ps,
        ):
            # ---- load constants / inputs ----
            ct = {}
            for name in inp:
                t = cp.tile(list(inp[name].shape), F32, tag=name)
                nc.sync.dma_start(t[:], inp[name][:])
                ct[name] = t
            x_nm, x_fm, qt_fm = ct["x_nm"], ct["x_fm"], ct["qt_fm"]
            iota8, i64x2, idn = ct["iota8"], ct["i64x2"], ct["idn"]
            ones1, ones64 = ct["ones1"], ct["ones64"]
            I64 = i64x2[:, 0:N_PER]
            W = (ct["Wk_all"], ct["W1_all"], ct["W2_all"], ct["W3_all"])

            # ---- persistent tiles ----
            cntw = [pp.tile([N_PER, 128], F32, tag=f"cntw{g}", name=f"cntw{g}")
                    for g in range(G)]
            A_g = [pp.tile([N_PER, N_PER], F32, tag=f"A{g}", name=f"Ag{g}")
                   for g in range(G)]
            S_g = [pp.tile([N_PER, N_PER], F32, tag=f"S{g}", name=f"Sg{g}")
                   for g in range(G)]
            dinv_all = pp.tile([1, NODES], F32, tag="dinv")
            xq_fm = pp.tile([H, NODES], F32, tag="xq_fm")
            xq2_fm = pp.tile([H, NODES], F32, tag="xq2_fm")
            xc_fm = pp.tile([H, NODES], F32, tag="xc_fm")
            xc_nm = pp.tile([N_PER, G * H], F32, tag="xc_nm")
            agg_fm = pp.tile([H, NODES], F32, tag="agg_fm")
            f1_sb = pp.tile([1, NODES], F32, tag="f1")
            f2_sb = pp.tile([1, NODES], F32, tag="f2")
            g1_sb = pp.tile([1, NODES], F32, tag="g1")
            g2_sb = pp.tile([1, NODES], F32, tag="g2")
            l1_sb = pp.tile([1, NODES], F32, tag="l1")
            l2_sb = pp.tile([1, NODES], F32, tag="l2")
            lf1_sb = pp.tile([1, NODES], F32, tag="lf1")
            lf2_sb = pp.tile([1, NODES], F32, tag="lf2")
            score_sb = pp.tile([1, NODES], F32, tag="score")

            # ================= PHASE 1: Cnt/Wsum build =================
            for g in range(G):
                ohs = []
                for half in range(2):
                    cs = g * NCHUNK + half * 8
                    soh = sb.tile([128, 512], F32, tag="soh")
                    ddw = sb.tile([128, 1024], F32, tag="ddw")
                    r3 = lambda t: t[:].rearrange("p (c n) -> p c n", n=N_PER)
                    d3 = ddw[:].rearrange("p (c t) -> p c t", t=128)
                    nc.vector.tensor_tensor(
                        r3(soh), r3(iota8),
                        ct["srcL"][:, cs:cs + 8][:, :, None]
                        .broadcast_to([128, 8, N_PER]), OP.is_equal)
                    nc.vector.tensor_tensor(
                        d3[:, :, 0:N_PER], r3(iota8),
                        ct["dstL"][:, cs:cs + 8][:, :, None]
                        .broadcast_to([128, 8, N_PER]), OP.is_equal)
                    nc.vector.tensor_tensor(
                        d3[:, :, N_PER:128], d3[:, :, 0:N_PER],
                        ct["wE"][:, cs:cs + 8][:, :, None]
                        .broadcast_to([128, 8, N_PER]), OP.mult)
                    ohs.append((soh, ddw))
                cw_ps = ps.tile([N_PER, 128], F32, tag="cw")
                for k in range(NCHUNK):
                    soh, ddw = ohs[k // 8]
                    j = (k % 8)
                    nc.tensor.matmul(cw_ps[:], soh[:, j * N_PER:(j + 1) * N_PER],
                                     ddw[:, j * 128:(j + 1) * 128],
                                     start=(k == 0), stop=(k == NCHUNK - 1))
                # add self loops to both Cnt and Wsum
                nc.vector.tensor_tensor(cntw[g][:], cw_ps[:], i64x2[:], OP.add)

            # ================= PHASE 2: deg, dinv, A =================
            deg_all = pp.tile([1, NODES], F32, tag="deg")
            for g in range(G):
                dg_ps = ps.tile([1, N_PER], F32, tag="sm")
                nc.tensor.matmul(dg_ps[:], ones64[:], cntw[g][:, N_PER:128],
                                 start=True, stop=True)
                nc.scalar.copy(deg_all[:, g * N_PER:(g + 1) * N_PER], dg_ps[:])
            _rsqrt(nc, sb, dinv_all, deg_all, "dinv")
            for g in range(G):
                gs = slice(g * N_PER, (g + 1) * N_PER)
                # dinv as column [64,1]
                dc_ps = ps.tile([N_PER, 1], F32, tag="sm")
                nc.tensor.matmul(dc_ps[:], dinv_all[:, gs], ones1[:, 0:1],
                                 start=True, stop=True)
                dcol = sb.tile([N_PER, 1], F32, tag="dcol")
                nc.scalar.copy(dcol[:], dc_ps[:])
                # dinv broadcast rows [64,64]
                db_ps = ps.tile([N_PER, N_PER], F32, tag="sm")
                nc.tensor.matmul(db_ps[:], ones1[:, 0:N_PER], dinv_all[:, gs],
                                 start=True, stop=True)
                tmp = sb.tile([N_PER, N_PER], F32, tag="atmp")
                nc.vector.tensor_scalar(tmp[:], cntw[g][:, N_PER:128], dcol[:],
                                        None, OP.mult)
                nc.vector.tensor_tensor(A_g[g][:], tmp[:], db_ps[:], OP.mult)

            # ================= PHASE 3: x_q = hop(hop(x)) =================
            def hops(v_nm_tile, out_fm, tag):
                # two propagation hops + transpose to feature-major
                for g in range(G):
                    gc = slice(g * H, (g + 1) * H)
                    h1_ps = ps.tile([N_PER, H], F32, tag=f"hop_ps")
                    nc.tensor.matmul(h1_ps[:], A_g[g][:], v_nm_tile[:, gc],
                                     start=True, stop=True)
                    h1_sb = sb.tile([N_PER, H], F32, tag=f"hop_sb")
                    nc.scalar.copy(h1_sb[:], h1_ps[:])
                    h2_ps = ps.tile([N_PER, H], F32, tag=f"hop_ps")
                    nc.tensor.matmul(h2_ps[:], A_g[g][:], h1_sb[:],
                                     start=True, stop=True)
                    h2_sb = sb.tile([N_PER, H], F32, tag=f"hop_sb")
                    nc.scalar.copy(h2_sb[:], h2_ps[:])
                    # transpose [64,128] -> [128,64] into out_fm slice
                    t_ps = ps.tile([H, N_PER], F32, tag="sm")
                    nc.tensor.transpose(t_ps[:], h2_sb[:], I64)
                    nc.scalar.copy(out_fm[:, g * N_PER:(g + 1) * N_PER], t_ps[:])

            hops(x_nm, xq_fm, "xq")

            # ================= PHASE 4: f1, f2 attentions =================
            _attention(nc, sb, ps, ct, x_fm, xq_fm, W, 0, f1_sb, lf1_sb)
            _attention(nc, sb, ps, ct, x_fm, qt_fm, W, 1, f2_sb, lf2_sb)

            # ================= PHASE 5: E, S, agg, x_c =================
            for g in range(G):
                gs = slice(g * N_PER, (g + 1) * N_PER)
                # f1 as column [64,1]
                f1c_ps = ps.tile([N_PER, 1], F32, tag="sm")
                nc.tensor.matmul(f1c_ps[:], f1_sb[:, gs], ones1[:, 0:1],
                                 start=True, stop=True)
                f1c = sb.tile([N_PER, 1], F32, tag="f1c")
                nc.scalar.copy(f1c[:], f1c_ps[:])
                # f2 broadcast rows, then E_T = Lrelu(f2[r] + f1[c]) in [c,r]
                f2b_ps = ps.tile([N_PER, N_PER], F32, tag="sm")
                nc.tensor.matmul(f2b_ps[:], ones1[:, 0:N_PER], f2_sb[:, gs],
                                 start=True, stop=True)
                eT = sb.tile([N_PER, N_PER], F32, tag="eT")
                nc.vector.tensor_scalar(eT[:], f2b_ps[:], f1c[:], None, OP.add)
                ee = sb.tile([N_PER, N_PER], F32, tag="ee")
                nc.scalar.activation(ee[:], eT[:], AF.Exp)
                # Cnt^T
                cT_ps = ps.tile([N_PER, N_PER], F32, tag="sm")
                nc.tensor.transpose(cT_ps[:], cntw[g][:, 0:N_PER], I64)
                U = sb.tile([N_PER, N_PER], F32, tag="U")
                nc.vector.tensor_tensor(U[:], ee[:], cT_ps[:], OP.mult)
                dcol2 = sb.tile([N_PER, 1], F32, tag="dcol2")
                nc.vector.tensor_reduce(dcol2[:], U[:], AX.X, OP.add)
                rcol = sb.tile([N_PER, 1], F32, tag="rcol")
                _recip(nc, sb, rcol, dcol2, "rcol")
                sT = sb.tile([N_PER, N_PER], F32, tag="sT")
                nc.vector.tensor_scalar(sT[:], U[:], rcol[:], None, OP.mult)
                s_ps = ps.tile([N_PER, N_PER], F32, tag="sm")
                nc.tensor.transpose(s_ps[:], sT[:], I64)
                nc.scalar.copy(S_g[g][:], s_ps[:])
                # agg[c,f] = sum_r S[r,c] x[r,f]
                ag_ps = ps.tile([N_PER, H], F32, tag="sm")
                nc.tensor.matmul(ag_ps[:], S_g[g][:], x_nm[:, g * H:(g + 1) * H],
                                 start=True, stop=True)
                ag_sb = sb.tile([N_PER, H], F32, tag="hop_sb")
                nc.scalar.copy(ag_sb[:], ag_ps[:])
                agT_ps = ps.tile([H, N_PER], F32, tag="sm")
                nc.tensor.transpose(agT_ps[:], ag_sb[:], I64)
                nc.scalar.copy(agg_fm[:, gs], agT_ps[:])

            # x_c = 0.5*(lrelu(x + agg@lin0) + lrelu(x + agg@lin1))
            th = []
            for h in range(2):
                ml_ps = ps.tile([H, NODES], F32, tag="mlp")
                nc.tensor.matmul(ml_ps[:], ct["lin_all"][:, h * 128:(h + 1) * 128],
                                 agg_fm[:], start=True, stop=True)
                t_sb = sb.tile([H, NODES], F32, tag=f"xc_t{h}")
                nc.vector.tensor_tensor(t_sb[:], ml_ps[:], x_fm[:], OP.add)
                t2_sb = sb.tile([H, NODES], F32, tag=f"xc_u{h}")
                _lrelu(nc, sb, t2_sb, t_sb, f"xc{h}")
                th.append(t2_sb)
            xc_t = sb.tile([H, NODES], F32, tag="xc_sum")
            nc.vector.tensor_tensor(xc_t[:], th[0][:], th[1][:], OP.add)
            nc.vector.tensor_scalar(xc_fm[:], xc_t[:], 0.5, None, OP.mult)
            # node-major copy of x_c for the second hops
            for g in range(G):
                gs = slice(g * N_PER, (g + 1) * N_PER)
                tn_ps = ps.tile([N_PER, H], F32, tag="sm")
                nc.tensor.transpose(tn_ps[:], xc_fm[:, gs], idn[:])
                nc.scalar.copy(xc_nm[:, g * H:(g + 1) * H], tn_ps[:])

            # ================= PHASE 6: second hops + g1/g2 =================
            hops(xc_nm, xq2_fm, "xq2")
            _attention(nc, sb, ps, ct, xc_fm, xq2_fm, W, 2, g1_sb, l1_sb)
            _attention(nc, sb, ps, ct, xc_fm, qt_fm, W, 3, g2_sb, l2_sb)

            # cluster score = per-graph softmax of (g1+g2), with max-subtract
            ssum = sb.tile([1, NODES], F32, tag="ssum")
            nc.vector.tensor_tensor(ssum[:], g1_sb[:], g2_sb[:], OP.add)
            s3 = ssum[:].rearrange("p (g n) -> p g n", n=N_PER)
            m8 = sb.tile([1, G], F32, tag="cs_m8")
            nc.vector.tensor_reduce(m8[:], s3, AX.X, OP.max)
            sm = sb.tile([1, NODES], F32, tag="cs_sm")
            nc.vector.tensor_tensor(
                sm[:].rearrange("p (g n) -> p g n", n=N_PER), s3,
                m8[:, :, None].broadcast_to([1, G, N_PER]), OP.subtract)
            e_cs = sb.tile([1, NODES], F32, tag="cs_e")
            nc.scalar.activation(e_cs[:], sm[:], AF.Exp)
            d8 = sb.tile([1, G], F32, tag="cs_d8")
            nc.vector.tensor_reduce(d8[:], e_cs[:].rearrange("p (g n) -> p g n",
                                                             n=N_PER), AX.X, OP.add)
            r8 = sb.tile([1, G], F32, tag="cs_r8")
            _recip(nc, sb, r8, d8, "cs_r8")
            nc.vector.tensor_tensor(
                score_sb[:].rearrange("p (g n) -> p g n", n=N_PER),
                e_cs[:].rearrange("p (g n) -> p g n", n=N_PER),
                r8[:, :, None].broadcast_to([1, G, N_PER]), OP.mult)

            # ================= PHASE 7: outputs =================
            xs_sb = pp.tile([N_PER, G * H], F32, tag="xs")
            for g in range(G):
                gs = slice(g * N_PER, (g + 1) * N_PER)
                sc_ps = ps.tile([N_PER, 1], F32, tag="sm")
                nc.tensor.matmul(sc_ps[:], score_sb[:, gs], ones1[:, 0:1],
                                 start=True, stop=True)
                sc_col = sb.tile([N_PER, 1], F32, tag="sc_col")
                nc.scalar.copy(sc_col[:], sc_ps[:])
                nc.vector.tensor_scalar(xs_sb[:, g * H:(g + 1) * H],
                                        x_nm[:, g * H:(g + 1) * H],
                                        sc_col[:], None, OP.mult)
                # Atilde = S^T (A S)
                aT_ps = ps.tile([N_PER, N_PER], F32, tag="sm")
                nc.tensor.transpose(aT_ps[:], A_g[g][:], I64)
                aT_sb = sb.tile([N_PER, N_PER], F32, tag="aT")
                nc.scalar.copy(aT_sb[:], aT_ps[:])
                p1_ps = ps.tile([N_PER, N_PER], F32, tag="sm")
                nc.tensor.matmul(p1_ps[:], aT_sb[:], S_g[g][:],
                                 start=True, stop=True)
                p1_sb = sb.tile([N_PER, N_PER], F32, tag="p1")
                nc.scalar.copy(p1_sb[:], p1_ps[:])
                at2_ps = ps.tile([N_PER, N_PER], F32, tag="sm")
                nc.tensor.matmul(at2_ps[:], S_g[g][:], p1_sb[:],
                                 start=True, stop=True)
                at2_sb = sb.tile([N_PER, N_PER], F32, tag="at2")
                nc.scalar.copy(at2_sb[:], at2_ps[:])
                nc.sync.dma_start(out["at2_out"][g], at2_sb[:])

            nc.sync.dma_start(out["xs_out"][:], xs_sb[:])
            nc.sync.dma_start(out["l1_out"][:], l1_sb[:])
            nc.sync.dma_start(out["l2_out"][:], l2_sb[:])
            nc.sync.dma_start(out["score_out"][:], score_sb[:])
            if debug:
                for g in range(G):
                    nc.sync.dma_start(out["dbg_cw"][:, g * 128:(g + 1) * 128],
                                      cntw[g][:])
                    nc.sync.dma_start(
                        out["dbg_a"][:, g * N_PER:(g + 1) * N_PER], A_g[g][:])
                    nc.sync.dma_start(
                        out["dbg_s"][:, g * N_PER:(g + 1) * N_PER], S_g[g][:])
                nc.sync.dma_start(out["dbg_xq"][:], xq_fm[:])
                nc.sync.dma_start(out["dbg_xc"][:], xc_fm[:])
                nc.sync.dma_start(out["dbg_f1"][:], f1_sb[:])
                nc.sync.dma_start(out["dbg_f2"][:], f2_sb[:])
                nc.sync.dma_start(out["dbg_dinv"][:], dinv_all[:])

    nc.compile()
    return nc


def get_nc(debug=False):
    key = bool(debug)
    if key not in _NC_CACHE:
        _NC_CACHE[key] = _build_nc(debug)
    return _NC_CACHE[key]


def _prep_in_maps(x, edge_index, edge_weight, target_x):
    x = np.asarray(x, np.float32)
    src = np.asarray(edge_index[0])
    dst = np.asarray(edge_index[1])
    w = np.asarray(edge_weight, np.float32)
    tx = np.asarray(target_x, np.float32)

    iota8 = np.tile(np.arange(N_PER, dtype=np.float32), (128, 8))
    i64x2 = np.concatenate([np.eye(N_PER, dtype=np.float32)] * 2, axis=1)
    idn = np.eye(128, dtype=np.float32)
    ones1 = np.ones((1, 128), np.float32)
    ones64 = np.ones((N_PER, 1), np.float32)

    in_maps = []
    for c in range(NCORES):
        xc = x[c * NODES:(c + 1) * NODES]
        x_nm = np.ascontiguousarray(
            xc.reshape(G, N_PER, H).transpose(1, 0, 2).reshape(N_PER, G * H))
        x_fm = np.ascontiguousarray(xc.T)
        qt_fm = np.ascontiguousarray(
            np.repeat(tx[c * G:(c + 1) * G], N_PER, axis=0).T)
        srcL = np.empty((128, G * NCHUNK), np.float32)
        dstL = np.empty((128, G * NCHUNK), np.float32)
        wEm = np.empty((128, G * NCHUNK), np.float32)
        for j in range(G):
            gg = c * G + j
            sl = slice(gg * EPG, (gg + 1) * EPG)
            cols = slice(j * NCHUNK, (j + 1) * NCHUNK)
            srcL[:, cols] = (src[sl] - gg * N_PER).reshape(NCHUNK, 128).T
            dstL[:, cols] = (dst[sl] - gg * N_PER).reshape(NCHUNK, 128).T
            wEm[:, cols] = w[sl].reshape(NCHUNK, 128).T
        in_maps.append(dict(
            x_nm=x_nm, x_fm=x_fm, qt_fm=qt_fm, srcL=srcL, dstL=dstL, wE=wEm,
            iota8=iota8, i64x2=i64x2, idn=idn, ones1=ones1, ones64=ones64))
    return in_maps


def _prep_weights(att_Wk, att_W1, att_W2, att_W3, lin_W):
    att_Wk = np.asarray(att_Wk, np.float32)
    att_W1 = np.asarray(att_W1, np.float32)
    att_W2 = np.asarray(att_W2, np.float32)
    att_W3 = np.asarray(att_W3, np.float32)
    lin_W = np.asarray(lin_W, np.float32)
    Wk_all = np.concatenate([att_Wk[a] for a in range(4)], axis=1)
    blocks = []
    for a in range(4):
        for kc in range(4):
            for mc in range(2):
                blocks.append(att_W1[a][kc * 128:(kc + 1) * 128,
                                        mc * 128:(mc + 1) * 128])
    W1_all = np.concatenate(blocks, axis=1)
    blocks = []
    for a in range(4):
        for kc in range(2):
            blocks.append(att_W2[a][kc * 128:(kc + 1) * 128, :])
    W2_all = np.concatenate(blocks, axis=1)
    W3_all = np.concatenate([att_W3[a] for a in range(4)], axis=1)
    lin_all = np.concatenate([lin_W[h] for h in range(2)], axis=1)
    return dict(Wk_all=np.ascontiguousarray(Wk_all),
                W1_all=np.ascontiguousarray(W1_all),
                W2_all=np.ascontiguousarray(W2_all),
                W3_all=np.ascontiguousarray(W3_all),
                lin_all=np.ascontiguousarray(lin_all))


def _softmax64_f64(l):
    l = np.asarray(l, np.float64).reshape(-1, N_PER)
    m = l.max(axis=1, keepdims=True)
    e = np.exp(l - m)
    return e / e.sum(axis=1, keepdims=True)


def kernel(x, edge_index, edge_weight, target_x, batch_ixs,
           att_Wk, att_W1, att_W2, att_W3, lin_W, _debug=False, _results=None,
           _trace=False, _tmpdir=None):
    nc = get_nc(_debug)
    in_maps = _prep_in_maps(x, edge_index, edge_weight, target_x)
    wmap = _prep_weights(att_Wk, att_W1, att_W2, att_W3, lin_W)
    for m in in_maps:
        m.update(wmap)
    res = run_bass_kernel_spmd(nc, in_maps, core_ids=list(range(NCORES)),
                               trace=_trace, tmpdir=_tmpdir)
    if _results is not None:
        _results.append(res)

    x = np.asarray(x, np.float32)
    batch_ixs = np.asarray(batch_ixs)
    Nk = B * KK
    x_out = np.empty((Nk, H), np.float32)
    A2 = np.zeros((Nk, Nk), np.float32)
    perm = np.empty(Nk, np.int32)
    for c in range(NCORES):
        r = res.results[c]
        xs = r["xs_out"]            # [64, G*128]
        at2 = r["at2_out"]          # [G, 64, 64]
        s = _softmax64_f64(r["l1_out"][0]) + _softmax64_f64(r["l2_out"][0])
        for j in range(G):
            gg = c * G + j
            idx = np.argsort(-s[j], kind="stable")[:KK].astype(np.int64)
            rows = slice(gg * KK, (gg + 1) * KK)
            perm[rows] = (idx + gg * N_PER).astype(np.int32)
            x_out[rows] = xs[:, j * H:(j + 1) * H][idx]
            A2[rows, rows] = at2[j][np.ix_(idx, idx)]
    A2[np.arange(Nk), np.arange(Nk)] = 1.0
    batch_out = batch_ixs[perm]
    return x_out, A2, batch_out, perm
